# revision 3
# baseline (speedup 1.0000x reference)
"""GAT (2-layer, 4-head) + MLP/BatchNorm predictor on 8 Trainium2 NeuronCores.

v2 redesign (vs baseline):
  - Table rows trimmed 768B -> 512B (just h~ = (x@W)*al in fp16). el is
    recomputed on-chip from gathered rows (sum over d); er for own dsts is
    computed by 49 tiny matmuls against the Wer columns (from a transposed
    own-features host param for L1, from the SBUF-resident x2acc for L2).
  - Projection uses DMA-transpose loads (no PE transposes), groups of 8
    tiles per DMA, ACT-engine PSUM->SBUF copies, one store DMA per group.
  - DLR mask source is int8 (halves the mask-broadcast HBM traffic).
  - IDX/DLC loaded into SBUF once, reused by both layers.
  - x2/x3 kept transposed ([D, nodes]) end to end: edge phase transposes
    its 128-node output tile on the PE, layer-2 projection reads plain
    strided slices of the AllGathered [8, D, NSHARD] buffer, and the MLP
    runs entirely in feature-major layout with zero transposes.
  - MLP: one pass computes z^T blocks (matmul + fused bias+relu on ACT),
    stats via ACT accum / DVE reduces, ONE AllReduce (var = E[z^2]-E[z]^2),
    BN folded into the final 2-col matmul with a ones-row bias trick.
"""
import sys

sys.path.insert(0, "/opt/trn_rl_repo")

import numpy as np

N = 50000
F_IN = 128
H = 4
D = 64
HD = 256
NCORES = 8
P = 128
NSHARD = N // NCORES            # 6250
NCHUNK = (NSHARD + P - 1) // P  # 49
SHPAD = NCHUNK * P              # 6272
NTILE = (N + P - 1) // P        # 391
NPAD = NTILE * P                # 50048
SPLIT = 32768
GPROJ = 16                      # projection tiles per DMA group
ROW = 384                       # table row: 256 h fp16 | 4 el f32 | pad -> 768B
MLP_H = 200
NCLS = 2
NEG = 0.2
EPS = 1e-5
MLP_B = MLP_H - P               # 72


def configure(n, split=32768):
    """Override problem size (simulator debugging)."""
    global N, NSHARD, NCHUNK, SHPAD, NTILE, NPAD, SPLIT
    N = n
    NSHARD = N // NCORES
    NCHUNK = (NSHARD + P - 1) // P
    SHPAD = NCHUNK * P
    NTILE = (N + P - 1) // P
    NPAD = NTILE * P
    SPLIT = split


# ----------------------------------------------------------------------------
# Host-side preprocessing
# ----------------------------------------------------------------------------

def _fold_weights(W, al, ar):
    """W:[F,H*D] al,ar:[H,D] -> Wext [F, HD+8] f16 = [W | Wel | Wer]."""
    F = W.shape[0]
    Wel = (W.reshape(F, H, D).astype(np.float64) * al[None]).sum(-1)
    Wer = (W.reshape(F, H, D).astype(np.float64) * ar[None]).sum(-1)
    return np.concatenate([W.astype(np.float64), Wel, Wer],
                          axis=1).astype(np.float16)


def _prep_edges(src, dst):
    """Per-core gather/mask arrays.

    Returns (plan, per_core):
      plan: T_lo[j], T_hi[j], totT (identical across cores).
      per-core: IDX [128, 8*totT] i16, DLC [128, totT] f32,
                DLR8 [1, totT*128] i8.
    """
    src = np.asarray(src)
    dst = np.asarray(dst)
    per_core = []
    for c in range(NCORES):
        m = (dst >= c * NSHARD) & (dst < (c + 1) * NSHARD)
        es, ed = src[m], dst[m] - c * NSHARD
        order = np.argsort(ed, kind="stable")
        es, ed = es[order], ed[order]
        starts = np.searchsorted(ed, np.arange(0, NCHUNK * P, P))
        ends = np.searchsorted(ed, np.minimum(np.arange(P, (NCHUNK + 1) * P, P), NSHARD))
        chunks = []
        for j in range(NCHUNK):
            cs, ce = starts[j], ends[j]
            s_j, d_j = es[cs:ce], ed[cs:ce] - j * P
            lo = s_j < SPLIT
            chunks.append((s_j[lo], d_j[lo], s_j[~lo] - SPLIT, d_j[~lo]))
        per_core.append(chunks)

    T_lo = np.zeros(NCHUNK, np.int64)
    T_hi = np.zeros(NCHUNK, np.int64)
    for c in range(NCORES):
        for j in range(NCHUNK):
            slo, _, shi, _ = per_core[c][j]
            T_lo[j] = max(T_lo[j], -(-len(slo) // P))
            T_hi[j] = max(T_hi[j], -(-len(shi) // P))
    T_lo = np.maximum(T_lo, 1)  # self-loops make lo nonzero anyway
    totT = int((T_lo + T_hi).sum())

    def wrap_idx(flat):
        n = len(flat)
        a = flat.reshape(n // 16, 16).T.astype(np.int16)   # [16, cols]
        return np.tile(a, (8, 1))                          # [128, cols]

    arrays = []
    for c in range(NCORES):
        idx_cols = []
        dlc = np.zeros((P, totT), np.float32)
        dlr = np.zeros(totT * P, np.int8)
        t0 = 0
        for j in range(NCHUNK):
            slo, dlo, shi, dhi = per_core[c][j]
            for (s_j, d_j, T) in ((slo, dlo, T_lo[j]), (shi, dhi, T_hi[j])):
                nslot = int(T) * P
                if nslot == 0:
                    continue
                idx = np.zeros(nslot, np.int16)
                dl = np.full(nslot, -1.0, np.float32)
                idx[: len(s_j)] = s_j
                dl[: len(s_j)] = d_j
                idx_cols.append(wrap_idx(idx))
                dlm = dl.reshape(int(T), P)                 # [t, p]
                dlc[:, t0 : t0 + int(T)] = dlm.T
                dlr[t0 * P : (t0 + int(T)) * P] = dl.astype(np.int8)
                t0 += int(T)
        assert t0 == totT
        IDX = np.concatenate(idx_cols, axis=1)
        assert IDX.shape == (P, 8 * totT)
        # m_all mask, host-built: MAM[p, t*128 + c] = (dlc[p, t] == c)
        import ml_dtypes
        f8 = ml_dtypes.float8_e4m3fn
        mam = (dlc[:, :, None] == np.arange(P, dtype=np.float32)[None, None, :])
        mam = mam.astype(f8).reshape(P, totT * P)
        # mt mask: MTM[c, t*128 + p] = (dlc[p, t] == c)
        dlrf = dlc.T.reshape(1, totT * P)   # [1, (t, p)] dst-local per slot
        mtm = (dlrf == np.arange(P, dtype=np.float32)[:, None]).astype(f8)
        arrays.append((IDX, mam, mtm))

    plan = {"T_lo": T_lo.tolist(), "T_hi": T_hi.tolist(), "totT": totT}
    return plan, arrays


def _proj2_segments():
    """Per projection group: (i0, g, segs, tail) with segs = list of
    (xT-col-offset, core, col0, length) mapping global rows to blocks of
    the [NCORES, D, NSHARD] AllGathered tensor; tail = trailing pad rows."""
    groups = []
    i = 0
    while i < NTILE:
        g = min(GPROJ, NTILE - i)
        r0, r1 = i * P, min((i + g) * P, N)
        segs = []
        r = r0
        while r < r1:
            c = r // NSHARD
            col0 = r - c * NSHARD
            ln = min(r1, (c + 1) * NSHARD) - r
            segs.append((r - r0, c, col0, ln))
            r += ln
        groups.append((i, g, segs, (i + g) * P - r1))
        i += g
    return groups


# ----------------------------------------------------------------------------
# Bass program
# ----------------------------------------------------------------------------

def _dram_ap(bass, t, offset, ap):
    base = t[:]
    return bass.AP(tensor=base.tensor, offset=offset, ap=ap)


def build_nc(plan, phases="full", taps=False, max_chunks=None, ev=3, reps=1):
    import concourse.bacc as bacc
    import concourse.bass as bass
    import concourse.tile as tile
    from concourse import mybir

    FP16 = mybir.dt.float16
    FP8 = mybir.dt.float8e4
    F32 = mybir.dt.float32
    I16 = mybir.dt.int16
    I8 = mybir.dt.int8
    ALU = mybir.AluOpType
    ACTF = mybir.ActivationFunctionType

    T_lo, T_hi, totT = plan["T_lo"], plan["T_hi"], plan["totT"]

    nc = bacc.Bacc("TRN2", target_bir_lowering=False, debug=False,
                   num_devices=NCORES)

    dp = lambda name, shape, dt: nc.declare_dram_parameter(name, shape, dt, isOutput=False)
    features = dp("features", [NPAD, F_IN], FP16)
    FOWNT = dp("FOWNT", [F_IN, SHPAD], FP16)
    IDX = dp("IDX", [P, 8 * totT], I16)
    MAM = dp("MAM", [P, totT * P], FP8)
    MTM = dp("MTM", [P, totT * P], FP8)
    IDENT = dp("IDENT", [P, P], FP16)
    W1EXT = dp("W1EXT", [F_IN, HD + 8], FP16)
    W2EXT = dp("W2EXT", [D, HD + 8], FP16)
    B1 = dp("B1", [P, HD], F32)
    B2 = dp("B2", [P, HD], F32)
    WM1 = dp("WM1", [D, MLP_H], FP16)
    BM1A = dp("BM1A", [P, 1], F32)
    BM1B = dp("BM1B", [P, 1], F32)
    NEGC0A = dp("NEGC0A", [P, 1], F32)
    NEGC0B = dp("NEGC0B", [P, 1], F32)
    WM2A = dp("WM2A", [P, NCLS], F32)
    WM2B = dp("WM2B", [P, NCLS], F32)
    GAMB = dp("GAMB", [P, 2], F32)
    BETB = dp("BETB", [P, 2], F32)
    BM2 = dp("BM2", [1, NCLS], F32)

    out = nc.declare_dram_parameter("out", [NSHARD, NCLS], F32, isOutput=True)

    table1 = nc.dram_tensor("table1", [NPAD, ROW], FP16)
    table2 = nc.dram_tensor("table2", [NPAD, ROW], FP16)
    x2sT = nc.dram_tensor("x2sT", [D, NSHARD], FP16)
    x2fT = nc.dram_tensor("x2fT", [NCORES, D, NSHARD], FP16, addr_space="Shared")
    ccin = nc.dram_tensor("ccin", [P, 4], F32)
    ccout = nc.dram_tensor("ccout", [P, 4], F32, addr_space="Shared")

    groups2 = _proj2_segments()

    blocks = []
    b0 = 0
    while b0 < NSHARD:
        blocks.append((b0, min(512, NSHARD - b0)))
        b0 += 512
    NBLK = len(blocks)
    full_j = NSHARD // P
    rem = NSHARD - full_j * P

    with tile.TileContext(nc) as tc:
        import contextlib
        with contextlib.ExitStack() as ctx:
            singles = ctx.enter_context(tc.tile_pool(name="singles", bufs=1))

            def load_const(param, shape, dt, tag):
                t = singles.tile(shape, dt, tag=tag)
                nc.sync.dma_start(out=t[:], in_=param[:])
                return t

            ident = load_const(IDENT, [P, P], FP16, "c_ident")
            w1ext = load_const(W1EXT, [F_IN, HD + 8], FP16, "c_w1ext")
            w2ext = load_const(W2EXT, [D, HD + 8], FP16, "c_w2ext")
            b1 = load_const(B1, [P, HD], F32, "c_b1")
            b2 = load_const(B2, [P, HD], F32, "c_b2")
            wm1 = load_const(WM1, [D, MLP_H], FP16, "c_wm1")
            bm1a = load_const(BM1A, [P, 1], F32, "c_bm1a")
            bm1b = load_const(BM1B, [P, 1], F32, "c_bm1b")
            negc0a = load_const(NEGC0A, [P, 1], F32, "c_negc0a")
            negc0b = load_const(NEGC0B, [P, 1], F32, "c_negc0b")
            wm2a = load_const(WM2A, [P, NCLS], F32, "c_wm2a")
            wm2b = load_const(WM2B, [P, NCLS], F32, "c_wm2b")
            gamb = load_const(GAMB, [P, 2], F32, "c_gamb")
            betb = load_const(BETB, [P, 2], F32, "c_betb")
            bm2 = load_const(BM2, [1, NCLS], F32, "c_bm2")
            fownt = load_const(FOWNT, [F_IN, SHPAD], FP16, "c_fownt")
            idx_all = load_const(IDX, [P, 8 * totT], I16, "c_idx")

            def _run_once():
                # ------------- projection phase -------------
                def projection(wext, F, table, loader, group_list):
                    with tc.tile_pool(name="proj_sb", bufs=3) as sb, \
                         tc.tile_pool(name="proj_ps", bufs=4, space="PSUM") as ps:
                        for (i0, g, payload) in group_list:
                            xT = sb.tile([F, GPROJ * P], FP16, tag="xT")
                            loader(xT, i0, g, payload)
                            rowt = sb.tile([P, GPROJ, HD + 8], FP16, tag="rowt")
                            for k in range(g):
                                hp = ps.tile([P, HD + 4], F32, tag="hp")
                                nc.tensor.matmul(hp[:], lhsT=xT[:, k * P:(k + 1) * P],
                                                 rhs=wext[:, 0:HD + 4],
                                                 start=True, stop=True)
                                nc.scalar.activation(rowt[:, k, 0:HD], hp[:, 0:HD],
                                                     ACTF.Copy)
                                nc.vector.tensor_copy(
                                    out=rowt[:, k, HD:HD + 8].bitcast(F32),
                                    in_=hp[:, HD:HD + 4])
                            tab_ap = _dram_ap(bass, table, i0 * P * ROW,
                                              [[ROW, P], [P * ROW, g], [1, HD + 8]])
                            nc.sync.dma_start(out=tab_ap, in_=rowt[:, 0:g, :])

                def l1_load(xT, i0, g, payload):
                    nc.sync.dma_start_transpose(
                        out=xT[:, 0:g * P],
                        in_=features[i0 * P:(i0 + g) * P, :])

                def l2_load(xT, i0, g, payload):
                    segs, tail = payload
                    for (off, c, col0, ln) in segs:
                        src = _dram_ap(bass, x2fT, (c * D) * NSHARD + col0,
                                       [[NSHARD, D], [1, ln]])
                        nc.sync.dma_start(out=xT[0:D, off:off + ln], in_=src)
                    if tail:
                        nc.vector.memset(xT[0:D, g * P - tail:g * P], 0.0)

                def er_own(xt_slices, wext, tag):
                    """er[dst-local] for own nodes: 49 tiny matmuls."""
                    t = singles.tile([P, NCHUNK, 4], FP16, tag=tag)
                    with tc.tile_pool(name="erp", bufs=2, space="PSUM") as ep:
                        for j in range(NCHUNK):
                            e = ep.tile([P, 4], F32, tag="e")
                            nc.tensor.matmul(e[:], lhsT=xt_slices(j),
                                             rhs=wext[:, HD + 4:HD + 8],
                                             start=True, stop=True)
                            nc.vector.tensor_copy(out=t[:, j, :], in_=e[:])
                    return t

                # ------------- edge phase -------------
                def edge_phase(table, ero, bias_c, acc_dst):
                    with tc.tile_pool(name="eg", bufs=4) as eg, \
                         tc.tile_pool(name="em", bufs=4) as em, \
                         tc.tile_pool(name="es", bufs=4) as es, \
                         tc.tile_pool(name="eps", bufs=2, space="PSUM") as eps:
                        toff = 0
                        for j in range(NCHUNK):
                            Tl, Th = T_lo[j], T_hi[j]
                            T = Tl + Th
                            gbuf = eg.tile([P, T, ROW], FP16, tag="gbuf")
                            nc.gpsimd.dma_gather(
                                out_ap=gbuf[:, 0:Tl, :], in_ap=table[0:SPLIT, :],
                                idxs_ap=idx_all[:, 8 * toff:8 * (toff + Tl)],
                                num_idxs=P * Tl, num_idxs_reg=P * Tl,
                                elem_size=ROW, single_packet=False)
                            if Th:
                                nc.gpsimd.dma_gather(
                                    out_ap=gbuf[:, Tl:T, :],
                                    in_ap=table[SPLIT:NPAD, :],
                                    idxs_ap=idx_all[:, 8 * (toff + Tl):8 * (toff + T)],
                                    num_idxs=P * Th, num_idxs_reg=P * Th,
                                    elem_size=ROW, single_packet=False)
                            el = gbuf[:, :, HD:HD + 8].bitcast(F32)
                            mt = em.tile([P, T * P], FP8, tag="mt")
                            nc.sync.dma_start(
                                out=mt[:],
                                in_=MTM[:, toff * P:(toff + T) * P])
                            m_all = em.tile([P, T, P], FP8, tag="m_all")
                            nc.scalar.dma_start(
                                out=m_all[:],
                                in_=MAM[:, toff * P:(toff + T) * P])
                            # er per slot
                            erps = eps.tile([P, T * 4], F32, tag="erp")
                            for t in range(T):
                                nc.tensor.matmul(erps[:, t * 4:(t + 1) * 4],
                                                 lhsT=mt[:, t * P:(t + 1) * P],
                                                 rhs=ero[:, j, :], start=True, stop=True)
                            # e = el + er; leaky; exp (fp16 out)
                            e_sb = es.tile([P, T, 4], F32, tag="e_sb")
                            nc.vector.tensor_tensor(
                                out=e_sb[:], in0=el,
                                in1=erps[:].rearrange("p (t h) -> p t h", h=4),
                                op=ALU.add)
                            lr = es.tile([P, T, 4], F32, tag="lr")
                            nc.vector.tensor_scalar(out=lr[:], in0=e_sb[:], scalar1=NEG,
                                                    scalar2=None, op0=ALU.mult)
                            nc.vector.tensor_tensor(out=lr[:], in0=e_sb[:], in1=lr[:],
                                                    op=ALU.max)
                            exg = es.tile([P, T, 4], FP16, tag="exg")
                            nc.scalar.activation(exg[:], lr[:], ACTF.Exp)
                            # scale gathered rows by ex (broadcast over d)
                            gb0 = gbuf[:, 0, 0:HD]
                            hv_all = bass.AP(tensor=gb0.tensor, offset=gb0.offset,
                                             ap=[gb0.ap[0], [ROW, T], [D, H], [1, D]])
                            ex0 = exg[:, 0, 0:4]
                            ex_b = bass.AP(tensor=ex0.tensor, offset=ex0.offset,
                                           ap=[ex0.ap[0], [4, T], [1, 4], [0, D]])
                            nc.vector.tensor_tensor(out=hv_all, in0=hv_all, in1=ex_b,
                                                    op=ALU.mult)
                            # aggregate
                            agg = eps.tile([P, HD], F32, tag="agg")
                            sden = eps.tile([P, 4], F32, tag="sden")
                            for t in range(T):
                                st, sp = (t == 0), (t == T - 1)
                                nc.tensor.matmul(agg[:], lhsT=m_all[:, t, :],
                                                 rhs=gbuf[:, t, 0:HD], start=st, stop=sp)
                                nc.tensor.matmul(sden[:], lhsT=m_all[:, t, :],
                                                 rhs=exg[:, t, :], start=st, stop=sp)
                            srin = es.tile([P, 4], F32, tag="srin")
                            nc.vector.tensor_scalar(out=srin[:], in0=sden[:],
                                                    scalar1=1e-20, scalar2=None,
                                                    op0=ALU.add)
                            sr = es.tile([P, 4], F32, tag="sr")
                            nc.vector.reciprocal(sr[:], srin[:])
                            osb = es.tile([P, HD], F32, tag="osb")
                            sr0 = sr[:, 0:1]
                            sr_b = bass.AP(tensor=sr0.tensor, offset=sr0.offset,
                                           ap=[sr0.ap[0], [1, 4], [0, D]])
                            nc.vector.tensor_tensor(
                                out=osb[:].rearrange("p (h d) -> p h d", h=H),
                                in0=agg[:].rearrange("p (h d) -> p h d", h=H),
                                in1=sr_b, op=ALU.mult)
                            nc.vector.tensor_tensor(out=osb[:], in0=osb[:],
                                                    in1=bias_c[:], op=ALU.add)
                            nc.scalar.activation(osb[:], osb[:], ACTF.Relu)
                            xo = es.tile([P, D], F32, tag="xo")
                            ob0 = osb[:, 0:1]
                            osb_dh = bass.AP(tensor=ob0.tensor, offset=ob0.offset,
                                             ap=[ob0.ap[0], [1, D], [D, H]])
                            nc.vector.tensor_reduce(out=xo[:], in_=osb_dh,
                                                    axis=mybir.AxisListType.X,
                                                    op=ALU.add)
                            xo16 = es.tile([P, D], FP16, tag="xo16")
                            nc.scalar.activation(xo16[:], xo[:], ACTF.Copy, scale=0.25)
                            tp = eps.tile([D, P], FP16, tag="tp")
                            nc.tensor.transpose(out=tp[:], in_=xo16[:],
                                                identity=ident[:])
                            nc.scalar.activation(acc_dst[:, j, :], tp[:], ACTF.Copy)
                            toff += T

                # ------------------------------ go ------------------------------
                x2acc = singles.tile([D, NCHUNK, P], FP16, tag="c_x2acc")
                x3acc = singles.tile([D, NCHUNK, P], FP16, tag="c_x3acc")

                l1_groups = [(i, min(GPROJ, NTILE - i), None)
                             for i in range(0, NTILE, GPROJ)]
                projection(w1ext, F_IN, table1, l1_load, l1_groups)
                erown1 = er_own(lambda j: fownt[:, j * P:(j + 1) * P],
                                w1ext, "c_erown1")
                edge_phase(table1, erown1, b1, x2acc)

                nc.sync.dma_start(out=x2sT[:, 0:full_j * P],
                                  in_=x2acc[:, 0:full_j, :])
                if rem:
                    nc.sync.dma_start(out=x2sT[:, full_j * P:NSHARD],
                                      in_=x2acc[:, full_j, 0:rem])

                nc.gpsimd.collective_compute(
                    "AllGather", mybir.AluOpType.bypass,
                    replica_groups=[list(range(NCORES))],
                    ins=[x2sT[:]], outs=[x2fT[:]])

                projection(w2ext, D, table2, l2_load,
                           [(i0, g, (segs, tail))
                            for (i0, g, segs, tail) in groups2])
                erown2 = er_own(lambda j: x2acc[:, j, :], w2ext, "c_erown2")
                edge_phase(table2, erown2, b2, x3acc)
                if rem:
                    nc.vector.memset(x3acc[:, full_j, rem:P], 0.0)

                # ------------------------------ MLP -----------------------------
                with tc.tile_pool(name="mlp", bufs=1) as mp, \
                     tc.tile_pool(name="mlpp", bufs=2, space="PSUM") as mpp:
                    zts1 = mp.tile([P, SHPAD], FP16, tag="zts1")
                    zts2 = mp.tile([P, SHPAD], FP16, tag="zts2")
                    nc.vector.memset(zts2[:], 0.0)
                    nc.vector.memset(zts2[96:97, :], 1.0)
                    q1acc = mp.tile([P, NBLK], F32, tag="q1acc")
                    q2acc = mp.tile([P, NBLK], F32, tag="q2acc")
                    zqs = mp.tile([P, 512], F32, tag="zqs")
                    zqs2 = mp.tile([P, 512], F32, tag="zqs2")
                    x3v = x3acc[:].rearrange("d j p -> d (j p)")
                    for k, (c0, cl) in enumerate(blocks):
                        zp1 = mpp.tile([P, 512], F32, tag="zp1")
                        nc.tensor.matmul(zp1[:, 0:cl], lhsT=wm1[:, 0:P],
                                         rhs=x3v[:, c0:c0 + cl],
                                         start=True, stop=True)
                        nc.scalar.activation(zts1[:, c0:c0 + cl], zp1[:, 0:cl],
                                             ACTF.Relu, bias=bm1a[:])
                        nc.scalar.activation(zts1[:, c0:c0 + cl],
                                             zts1[:, c0:c0 + cl],
                                             ACTF.Identity, bias=negc0a[:])
                        zp2 = mpp.tile([P, 512], F32, tag="zp2")
                        nc.tensor.matmul(zp2[0:MLP_B, 0:cl], lhsT=wm1[:, P:MLP_H],
                                         rhs=x3v[:, c0:c0 + cl],
                                         start=True, stop=True)
                        nc.scalar.activation(zts2[0:MLP_B, c0:c0 + cl],
                                             zp2[0:MLP_B, 0:cl],
                                             ACTF.Relu, bias=bm1b[0:MLP_B, :])
                        nc.scalar.activation(zts2[0:MLP_B, c0:c0 + cl],
                                             zts2[0:MLP_B, c0:c0 + cl],
                                             ACTF.Identity, bias=negc0b[0:MLP_B, :])
                        nc.scalar.activation(zqs[:, 0:cl], zts1[:, c0:c0 + cl],
                                             ACTF.Square,
                                             accum_out=q1acc[:, k:k + 1])
                        nc.scalar.activation(zqs2[0:MLP_B, 0:cl],
                                             zts2[0:MLP_B, c0:c0 + cl],
                                             ACTF.Square,
                                             accum_out=q2acc[0:MLP_B, k:k + 1])
                    if NSHARD < SHPAD:
                        nc.vector.memset(zts1[:, NSHARD:SHPAD], 0.0)
                    pk = mp.tile([P, 4], F32, tag="pk")
                    nc.vector.memset(pk[:], 0.0)
                    nc.vector.tensor_reduce(out=pk[:, 0:1], in_=zts1[:, 0:NSHARD],
                                            axis=mybir.AxisListType.X, op=ALU.add)
                    nc.vector.tensor_reduce(out=pk[0:MLP_B, 1:2],
                                            in_=zts2[0:MLP_B, 0:NSHARD],
                                            axis=mybir.AxisListType.X, op=ALU.add)
                    nc.vector.tensor_reduce(out=pk[:, 2:3], in_=q1acc[:],
                                            axis=mybir.AxisListType.X, op=ALU.add)
                    nc.vector.tensor_reduce(out=pk[0:MLP_B, 3:4],
                                            in_=q2acc[0:MLP_B, :],
                                            axis=mybir.AxisListType.X, op=ALU.add)
                    nc.sync.dma_start(out=ccin[:], in_=pk[:])

                    nc.gpsimd.collective_compute(
                        "AllReduce", mybir.AluOpType.add,
                        replica_groups=[list(range(NCORES))],
                        ins=[ccin[:]], outs=[ccout[:]])

                    stg = mp.tile([P, 4], F32, tag="stg")
                    nc.sync.dma_start(out=stg[:], in_=ccout[:])
                    mus = mp.tile([P, 2], F32, tag="mus")
                    nc.vector.tensor_scalar(out=mus[:], in0=stg[:, 0:2],
                                            scalar1=1.0 / N, scalar2=None, op0=ALU.mult)
                    var = mp.tile([P, 2], F32, tag="var")
                    nc.vector.tensor_scalar(out=var[:], in0=stg[:, 2:4],
                                            scalar1=1.0 / N, scalar2=None, op0=ALU.mult)
                    musq = mp.tile([P, 2], F32, tag="musq")
                    nc.vector.tensor_tensor(out=musq[:], in0=mus[:], in1=mus[:],
                                            op=ALU.mult)
                    nc.vector.tensor_tensor(out=var[:], in0=var[:], in1=musq[:],
                                            op=ALU.subtract)
                    nc.vector.tensor_scalar(out=var[:], in0=var[:], scalar1=EPS,
                                            scalar2=None, op0=ALU.add)
                    std = mp.tile([P, 2], F32, tag="std")
                    nc.scalar.activation(std[:], var[:], ACTF.Sqrt)
                    rstd = mp.tile([P, 2], F32, tag="rstd")
                    nc.vector.reciprocal(rstd[:], std[:])
                    g2 = mp.tile([P, 2], F32, tag="g2")
                    nc.vector.tensor_tensor(out=g2[:], in0=gamb[:], in1=rstd[:],
                                            op=ALU.mult)
                    wg1 = mp.tile([P, NCLS], FP16, tag="wg1")
                    nc.vector.tensor_scalar_mul(wg1[:], wm2a[:], g2[:, 0:1])
                    wg2 = mp.tile([P, NCLS], FP16, tag="wg2")
                    nc.vector.memset(wg2[:], 0.0)
                    nc.vector.tensor_scalar_mul(wg2[0:MLP_B, :], wm2b[0:MLP_B, :],
                                                g2[0:MLP_B, 1:2])
                    bp = mp.tile([P, 2], F32, tag="bp")
                    nc.vector.tensor_tensor(out=bp[:], in0=mus[:], in1=g2[:],
                                            op=ALU.mult)
                    nc.vector.tensor_tensor(out=bp[:], in0=betb[:], in1=bp[:],
                                            op=ALU.subtract)
                    cp = mpp.tile([1, NCLS], F32, tag="cp")
                    nc.tensor.matmul(cp[:], lhsT=bp[:, 0:1], rhs=wm2a[:],
                                     start=True, stop=False)
                    nc.tensor.matmul(cp[:], lhsT=bp[0:MLP_B, 1:2],
                                     rhs=wm2b[0:MLP_B, :], start=False, stop=True)
                    cps = mp.tile([1, NCLS], F32, tag="cps")
                    nc.vector.tensor_tensor(out=cps[:], in0=cp[:], in1=bm2[:],
                                            op=ALU.add)
                    nc.scalar.activation(wg2[96:97, :], cps[:], ACTF.Copy)

                    oacc = mp.tile([P, NCHUNK, NCLS], F32, tag="oacc")
                    for j in range(NCHUNK):
                        op_ = mpp.tile([P, NCLS], F32, tag="op")
                        nc.tensor.matmul(op_[:], lhsT=zts1[:, j * P:(j + 1) * P],
                                         rhs=wg1[:], start=True, stop=False)
                        nc.tensor.matmul(op_[:],
                                         lhsT=zts2[0:97, j * P:(j + 1) * P],
                                         rhs=wg2[0:97, :],
                                         start=False, stop=True)
                        nc.vector.tensor_copy(out=oacc[:, j, :], in_=op_[:])
                    out_ap = _dram_ap(bass, out, 0,
                                      [[NCLS, P], [NCLS * P, full_j], [1, NCLS]])
                    nc.sync.dma_start(out=out_ap, in_=oacc[:, 0:full_j, :])
                    if rem:
                        tail_ap = _dram_ap(bass, out, full_j * P * NCLS,
                                           [[NCLS, rem], [1, NCLS]])
                        nc.sync.dma_start(out=tail_ap, in_=oacc[0:rem, full_j, :])

            for _rep in range(reps):
                _run_once()
    nc.finalize()
    return nc


# ----------------------------------------------------------------------------
# Host entry
# ----------------------------------------------------------------------------

def prep_inputs(inputs):
    f32 = np.float32
    W1ext = _fold_weights(np.asarray(inputs["W1"], f32),
                          np.asarray(inputs["al1"], f32),
                          np.asarray(inputs["ar1"], f32))
    W2ext = _fold_weights(np.asarray(inputs["W2"], f32),
                          np.asarray(inputs["al2"], f32),
                          np.asarray(inputs["ar2"], f32))
    plan, earrays = _prep_edges(inputs["src"], inputs["dst"])

    feats = np.asarray(inputs["features"], f32).astype(np.float16)
    features = np.zeros((NPAD, F_IN), np.float16)
    features[0:N] = feats
    ident = np.eye(P, dtype=np.float16)
    bc = lambda v: np.broadcast_to(np.asarray(v, f32).reshape(1, -1),
                                   (P, np.asarray(v).size)).copy()
    bm1 = np.asarray(inputs["bm1"], f32)
    wm2 = np.asarray(inputs["Wm2"], f32)
    gamma = np.asarray(inputs["gamma"], f32)
    beta = np.asarray(inputs["beta"], f32)
    colpad = lambda v: np.concatenate([v, np.zeros(P - v.size, f32)]).reshape(P, 1)
    rowpad = lambda m: np.concatenate(
        [m, np.zeros((P - m.shape[0], m.shape[1]), f32)], axis=0)
    gb = np.zeros((P, 2), f32)
    gb[:, 0] = gamma[0:P]
    gb[0:MLP_B, 1] = gamma[P:MLP_H]
    bt = np.zeros((P, 2), f32)
    bt[:, 0] = beta[0:P]
    bt[0:MLP_B, 1] = beta[P:MLP_H]
    consts = {
        "IDENT": ident,
        "W1EXT": W1ext, "W2EXT": W2ext,
        "B1": bc(inputs["b1"]), "B2": bc(inputs["b2"]),
        "WM1": np.asarray(inputs["Wm1"], np.float16),
        "BM1A": bm1[0:P].reshape(P, 1).astype(f32),
        "BM1B": colpad(bm1[P:MLP_H]),
        "NEGC0A": -np.maximum(bm1[0:P], 0).reshape(P, 1).astype(f32),
        "NEGC0B": colpad(-np.maximum(bm1[P:MLP_H], 0)),
        "WM2A": wm2[0:P].astype(f32),
        "WM2B": rowpad(wm2[P:MLP_H]),
        "GAMB": gb, "BETB": bt,
        "BM2": np.asarray(inputs["bm2"], f32).reshape(1, NCLS),
    }
    in_maps = []
    for c in range(NCORES):
        IDXa, MAMa, MTMa = earrays[c]
        m = dict(consts)
        m["features"] = features
        fown = np.zeros((SHPAD, F_IN), np.float16)
        fown[0:NSHARD] = feats[c * NSHARD:(c + 1) * NSHARD]
        m["FOWNT"] = np.ascontiguousarray(fown.T)
        m["IDX"] = IDXa
        m["MAM"] = MAMa
        m["MTM"] = MTMa
        in_maps.append(m)
    return plan, in_maps


def kernel(**inputs):
    from concourse.bass_utils import run_bass_kernel_spmd
    plan, in_maps = prep_inputs(inputs)
    nc = build_nc(plan)
    res = run_bass_kernel_spmd(nc, in_maps, core_ids=list(range(NCORES)))
    out = np.concatenate([res.results[c]["out"] for c in range(NCORES)], axis=0)
    return out.astype(np.float32)
